# revision 3
# baseline (speedup 1.0000x reference)
"""Trainium2 Bass kernel for the sparse-attention scores module.

Computes, for each batch b:
    scores[b, :] = softmax_s( v . tanh(W1 @ static[b] + W2 @ dynamic[b] + W3 @ hidden[b]) )
with W = [W1 | W2 | W3] of shape [H, 3H], static/dynamic [B, H, S], hidden [B, H].

Sharding: data-parallel over B across 8 NeuronCores (8 batches per core).

v2: the two big encoder tensors are quantized to fp8 e3m4 (4 mantissa bits) on
the host and laid out in DRAM exactly in tile order (pure linear DMA, 16
MiB/core, vs 64 MiB for fp32). W1/W2 are also e3m4 (scaled x64 to stay in
normal range; undone for free by the tanh's ACT scale operand). The matmul
streams at bf16 rate (1 col/cycle), but fp8 weights enable fast-weight-load so
the per-tile LDWEIGHTS hides under the matmul (the fp32r baseline serialized
~107ns of weight load per matmul). tanh outputs and v stay bf16 (negligible
error), as does the small on-device W3 @ hidden bias matmul. Measured end-to-end
rel l2 error vs the fp32 reference: ~1.05e-2.
"""

import sys

sys.path.insert(0, "/opt/trn_rl_repo")

import numpy as np
import ml_dtypes

B, H, S = 64, 256, 4096
N_CORES = 8
BPC = B // N_CORES          # batches per core
NCH = S // 512              # 8 psum column chunks
NQ = 2                      # input DMA quarters along s
SQ = S // NQ                # 2048 columns per quarter
SW = 64.0                   # fp8 scale on W1/W2
FP8 = ml_dtypes.float8_e3m4
BF16 = ml_dtypes.bfloat16


def build_bass(reps: int = 1, loop_iters: int = 0):
    """Build the per-core Bass program. reps>1 unrolls the whole computation
    multiple times; loop_iters>0 additionally wraps the unrolled body in a
    hardware loop. Both are used only for timing by differencing."""
    import contextlib

    import concourse.bacc as bacc
    import concourse.tile as tile
    from concourse import mybir

    f32 = mybir.dt.float32
    bf16 = mybir.dt.bfloat16
    f8 = mybir.dt.float8e3

    nc = bacc.Bacc(None)

    xq = nc.dram_tensor("xq", [BPC, 2, NQ, 128, 2, SQ], f8, kind="ExternalInput")
    wdr = nc.dram_tensor("wdr", [128, 2, 2, 2, 128], f8, kind="ExternalInput")
    vtb = nc.dram_tensor("vtb", [128, 2], bf16, kind="ExternalInput")
    w3t = nc.dram_tensor("w3t", [128, 2, 2, 128], bf16, kind="ExternalInput")
    ht = nc.dram_tensor("ht", [128, 2, BPC], bf16, kind="ExternalInput")
    out = nc.dram_tensor("out", [BPC, S], f32, kind="ExternalOutput")

    with tile.TileContext(nc) as tc:
        with (
            tc.tile_pool(name="consts", bufs=1) as consts,
            tc.tile_pool(name="xpool", bufs=2) as xpool,
            tc.tile_pool(name="tpool", bufs=6) as tpool,
            tc.tile_pool(name="spool", bufs=2) as spool,
            tc.tile_pool(name="mpsum", bufs=4, space="PSUM") as mpsum,
            tc.tile_pool(name="vpsum", bufs=2, space="PSUM") as vpsum,
            tc.tile_pool(name="spsum", bufs=2, space="PSUM") as spsum,
        ):
            wdr_sb = consts.tile([128, 2, 2, 2, 128], f8)
            nc.sync.dma_start(out=wdr_sb, in_=wdr[:, :, :, :, :])
            vtb_sb = consts.tile([128, 2], bf16)
            nc.sync.dma_start(out=vtb_sb, in_=vtb[:, :])
            w3t_sb = consts.tile([128, 2, 2, 128], bf16)
            nc.sync.dma_start(out=w3t_sb, in_=w3t[:, :, :, :])
            ht_sb = consts.tile([128, 2, BPC], bf16)
            nc.sync.dma_start(out=ht_sb, in_=ht[:, :, :])

            # Inline 0/1 masks for the softmax normalization matmuls:
            # bsum[b] = sum_n esums[8b+n]; brep[8b+n] = bsum[b].
            ma_np = np.zeros((64, BPC), np.float32)
            mb_np = np.zeros((BPC, 64), np.float32)
            for p in range(64):
                ma_np[p, p // NCH] = 1.0
                mb_np[p // NCH, p] = 1.0
            ma_dram = nc.inline_tensor(ma_np, name="ma")
            mb_dram = nc.inline_tensor(mb_np, name="mb")
            ma_sb = consts.tile([64, BPC], f32)
            nc.sync.dma_start(out=ma_sb, in_=ma_dram[:, :])
            mb_sb = consts.tile([BPC, 64], f32)
            nc.sync.dma_start(out=mb_sb, in_=mb_dram[:, :])

            # Per-batch bias: bias[m*128+c, b] = (W3 @ hidden[b])[m*128+c],
            # computed on device in bf16 (error ~1e-3 relative, negligible).
            bias_sb = consts.tile([128, 2, BPC], f32)
            for m in range(2):
                bias_ps = spsum.tile([128, BPC], f32, tag="small")
                for kk in range(2):
                    nc.tensor.matmul(
                        bias_ps,
                        lhsT=w3t_sb[:, kk, m, :],
                        rhs=ht_sb[:, kk, :],
                        start=(kk == 0),
                        stop=(kk == 1),
                    )
                nc.vector.tensor_copy(out=bias_sb[:, m, :], in_=bias_ps)

            loop_cm = (
                tc.For_i(0, loop_iters, 1) if loop_iters else contextlib.nullcontext()
            )
            with loop_cm:
              for _ in range(reps):
                # Scores live as [64, 512] with partition p = 8*b + n so the
                # epilogue runs on all 64 partitions at once.
                scores64 = spool.tile([64, 512], f32, tag="scores")
                pending = None

                def emit_vdot(pend):
                    # v-dot runs one chunk late so the tanh results are
                    # ready and the PE never waits on the ACT engine.
                    row, vp, tt = pend
                    for m in range(2):
                        nc.tensor.matmul(
                            vp,
                            lhsT=vtb_sb[:, m : m + 1],
                            rhs=tt[:, m, :],
                            start=(m == 0),
                            stop=(m == 1),
                        )
                    # Compute engines may only address partition bases
                    # that are multiples of 32, so the chunk is drained to
                    # partition 0 and a tiny SBUF->SBUF DMA places it at
                    # partition 8b+n of the scores tile.
                    stage = tpool.tile([1, 512], f32, tag="stage")
                    nc.vector.tensor_copy(out=stage, in_=vp)
                    nc.gpsimd.dma_start(
                        out=scores64[row : row + 1, :],
                        in_=stage,
                    )

                for b in range(BPC):
                    # Stream the two fp8 encoder tensors in 512 KiB halves;
                    # DRAM is already laid out in tile order so each DMA is
                    # a pure linear read.
                    xt = {}
                    for q in range(NQ):
                        for t in range(2):
                            xtile = xpool.tile([128, 2, SQ], f8, tag=f"x{t}{q}")
                            nc.sync.dma_start(out=xtile, in_=xq[b, t, q])
                            xt[t, q] = xtile

                    for n in range(NCH):
                        q, r = divmod(n, NCH // NQ)
                        tt = tpool.tile([128, 2, 512], bf16, tag="tt")
                        for m in range(2):
                            ps = mpsum.tile([128, 512], f32, tag="ps")
                            i = 0
                            for t in range(2):
                                for kk in range(2):
                                    nc.tensor.matmul(
                                        ps,
                                        lhsT=wdr_sb[:, t, m, kk, :],
                                        rhs=xt[t, q][:, kk, r * 512 : (r + 1) * 512],
                                        start=(i == 0),
                                        stop=(i == 3),
                                    )
                                    i += 1
                            nc.scalar.activation(
                                out=tt[:, m, :],
                                in_=ps,
                                func=mybir.ActivationFunctionType.Tanh,
                                bias=bias_sb[:, m, b : b + 1],
                                scale=1.0 / SW,
                            )
                        if pending is not None:
                            emit_vdot(pending)
                        vp = vpsum.tile([1, 512], f32, tag="vp")
                        pending = (b * NCH + n, vp, tt)
                # flush the last batch's final v-dot after the loop
                emit_vdot(pending)

                # Softmax epilogue. Scores are small (|s| < ~6), so skip the
                # max subtraction: softmax = exp(s) / sum(exp(s)). The
                # per-batch sums are formed from the per-partition accum via
                # two tiny 0/1-mask matmuls (sum over n, then broadcast).
                esums = spool.tile([64, 1], f32, tag="esums")
                nc.scalar.activation(
                    out=scores64,
                    in_=scores64,
                    func=mybir.ActivationFunctionType.Exp,
                    accum_out=esums,
                )
                bsum_ps = spsum.tile([BPC, 1], f32, tag="small")
                nc.tensor.matmul(bsum_ps, lhsT=ma_sb, rhs=esums,
                                 start=True, stop=True)
                bsum_sb = spool.tile([BPC, 1], f32, tag="bsum")
                nc.vector.tensor_copy(out=bsum_sb, in_=bsum_ps)
                brep_ps = spsum.tile([64, 1], f32, tag="small")
                nc.tensor.matmul(brep_ps, lhsT=mb_sb, rhs=bsum_sb,
                                 start=True, stop=True)
                recip = spool.tile([64, 1], f32, tag="recip")
                nc.vector.reciprocal(out=recip, in_=brep_ps)
                nc.vector.tensor_scalar_mul(out=scores64, in0=scores64, scalar1=recip)
                nc.gpsimd.dma_start(
                    out=out[:, :].rearrange("b (n s) -> (b n) s", n=NCH),
                    in_=scores64,
                )

    nc.finalize()
    return nc


def prep_shared_inputs(W: np.ndarray, v: np.ndarray, decoder_hidden: np.ndarray):
    """Host-side layout marshaling of the small replicated parameters."""
    W = np.ascontiguousarray(W, dtype=np.float32)
    # wdr[p, t, m, i, c] = SW * W[m*128+c, t*H + i*128+p], quantized to fp8.
    wdr = np.empty((128, 2, 2, 2, 128), FP8)
    for t in range(2):
        Wt = W[:, t * H : (t + 1) * H]  # [h, k]
        for m in range(2):
            for i in range(2):
                blk = Wt[m * 128 : (m + 1) * 128, i * 128 : (i + 1) * 128]  # [c, p]
                wdr[:, t, m, i, :] = (SW * blk.T).astype(FP8)
    # vtb[p, m] = v[m*128+p] in bf16
    vtb = np.ascontiguousarray(v[0].reshape(2, 128).T).astype(BF16)
    # w3t[p, kk, m, c] = W3[m*128+c, kk*128+p] in bf16 (unscaled)
    W3 = W[:, 2 * H : 3 * H]  # [h, k]
    w3t = np.empty((128, 2, 2, 128), BF16)
    for kk in range(2):
        for m in range(2):
            w3t[:, kk, m, :] = (
                W3[m * 128 : (m + 1) * 128, kk * 128 : (kk + 1) * 128]
                .T.astype(BF16)
            )
    hT = decoder_hidden[0].T.astype(np.float32)  # [H, B]
    return wdr, vtb, w3t, hT


def _tileize(x: np.ndarray) -> np.ndarray:
    """[B, H, S] fp32 -> fp8 [B, NQ, 128, 2, SQ] tile-order layout."""
    x8 = x.astype(FP8)
    return np.ascontiguousarray(
        x8.reshape(B, 2, 128, NQ, SQ).transpose(0, 3, 2, 1, 4)
    )


_CACHED = {}


def _get_nc(reps: int = 1, loop_iters: int = 0):
    key = (reps, loop_iters)
    if key not in _CACHED:
        _CACHED[key] = build_bass(reps, loop_iters)
    return _CACHED[key]


def make_in_maps(static_enc, dynamic_enc, decoder_hidden, W, v):
    wdr, vtb, w3t, hT = prep_shared_inputs(W, v, decoder_hidden)
    xs_t = _tileize(np.asarray(static_enc, dtype=np.float32))
    xd_t = _tileize(np.asarray(dynamic_enc, dtype=np.float32))
    # xq[b, t, q, p, i, s]
    xq_all = np.ascontiguousarray(np.stack([xs_t, xd_t], axis=1))
    in_maps = []
    for c in range(N_CORES):
        b0 = c * BPC
        ht_c = np.ascontiguousarray(
            hT[:, b0 : b0 + BPC].reshape(2, 128, BPC).transpose(1, 0, 2)
        ).astype(BF16)  # [p, kk, b]
        in_maps.append(
            {
                "xq": xq_all[b0 : b0 + BPC],
                "wdr": wdr,
                "vtb": vtb,
                "w3t": w3t,
                "ht": ht_c,
            }
        )
    return in_maps


def kernel(static_enc, dynamic_enc, decoder_hidden, W, v):
    from concourse.bass_utils import run_bass_kernel_spmd

    nc = _get_nc(reps=1)
    in_maps = make_in_maps(static_enc, dynamic_enc, decoder_hidden, W, v)
    res = run_bass_kernel_spmd(nc, in_maps, core_ids=list(range(N_CORES)))
    return np.concatenate([r["out"] for r in res.results], axis=0)


# revision 4
# speedup vs baseline: 1.5412x; 1.5412x over previous
"""Trainium2 Bass kernel for the sparse-attention scores module.

Computes, for each batch b:
    scores[b, :] = softmax_s( v . tanh(W1 @ static[b] + W2 @ dynamic[b] + W3 @ hidden[b]) )
with W = [W1 | W2 | W3] of shape [H, 3H], static/dynamic [B, H, S], hidden [B, H].

Sharding: data-parallel over B across 8 NeuronCores (8 batches per core).

v3: the two big encoder tensors are quantized to fp8 on the host and laid out
in DRAM in tile order (pure linear DMA, 16 MiB/core vs 64 for fp32). Output
columns are split by precision: the first DRC of 8 column chunks use fp8 e4m3
and run the [256,512] contraction in DoubleRow mode (2 fp8 weights per PE cell,
256-deep contraction per pass -> ~2x PE throughput); the rest use fp8 e3m4 (4
mantissa bits) at bf16 rate. The error contributions average across columns:
measured rel l2 vs the fp32 reference is ~1.6e-2 at DRC=4 (gate 2e-2).

The v-reduction uses the identity
    score[s] = sum_p v[p] * (t[p,s] + alpha[p] * t[128+p,s]),  alpha = v_hi/v_lo
so a single DVE multiply-add (on the otherwise idle Vector engine) folds the
256-partition reduction into 128, halving the v-dot matmul count on the PE.
W3 @ hidden is a tiny on-device bf16 matmul folded into the tanh via the ACT
per-partition bias; the fp8 weight scaling (x64) is undone by the ACT scale.
"""

import sys

sys.path.insert(0, "/opt/trn_rl_repo")

import numpy as np
import ml_dtypes

B, H, S = 64, 256, 4096
N_CORES = 8
BPC = B // N_CORES          # batches per core
NCH = S // 512              # 8 psum column chunks
SQ = 2048                   # columns per DMA tile (half of S)
DRC = 4                     # chunks (of 8) on the DoubleRow e4m3 path
SW = 64.0                   # fp8 scale on W1/W2
E4 = ml_dtypes.float8_e4m3  # TRN FP8_EXP4-compatible (max 240)
E3 = ml_dtypes.float8_e3m4
BF16 = ml_dtypes.bfloat16


def build_bass(reps: int = 1, loop_iters: int = 0):
    """Build the per-core Bass program. reps>1 unrolls the whole computation
    multiple times; loop_iters>0 additionally wraps the unrolled body in a
    hardware loop. Both are used only for timing by differencing."""
    import contextlib

    import concourse.bacc as bacc
    import concourse.tile as tile
    from concourse import mybir

    f32 = mybir.dt.float32
    f32r = mybir.dt.float32r
    bf16 = mybir.dt.bfloat16
    f8a = mybir.dt.float8e4
    f8b = mybir.dt.float8e3
    DR = mybir.MatmulPerfMode.DoubleRow

    nc = bacc.Bacc(None)

    xqa = nc.dram_tensor("xqa", [BPC, 2, 128, 2, SQ], f8a, kind="ExternalInput")
    xqb = nc.dram_tensor("xqb", [BPC, 2, 128, 2, SQ], f8b, kind="ExternalInput")
    wdra = nc.dram_tensor("wdra", [128, 2, 2, 2, 128], f8a, kind="ExternalInput")
    wdrb = nc.dram_tensor("wdrb", [128, 2, 2, 2, 128], f8b, kind="ExternalInput")
    vlo = nc.dram_tensor("vlo", [128, 1], f32r, kind="ExternalInput")
    alph = nc.dram_tensor("alph", [128, 1], f32, kind="ExternalInput")
    w3t = nc.dram_tensor("w3t", [128, 2, 2, 128], bf16, kind="ExternalInput")
    ht = nc.dram_tensor("ht", [128, 2, BPC], bf16, kind="ExternalInput")
    out = nc.dram_tensor("out", [BPC, S], f32, kind="ExternalOutput")

    with tile.TileContext(nc) as tc:
        with (
            tc.tile_pool(name="consts", bufs=1) as consts,
            tc.tile_pool(name="xpool", bufs=2) as xpool,
            tc.tile_pool(name="tpool", bufs=6) as tpool,
            tc.tile_pool(name="spool", bufs=2) as spool,
            tc.tile_pool(name="mpsum", bufs=4, space="PSUM") as mpsum,
            tc.tile_pool(name="vpsum", bufs=2, space="PSUM") as vpsum,
            tc.tile_pool(name="spsum", bufs=2, space="PSUM") as spsum,
        ):
            wdra_sb = consts.tile([128, 2, 2, 2, 128], f8a)
            nc.sync.dma_start(out=wdra_sb, in_=wdra[:, :, :, :, :])
            wdrb_sb = consts.tile([128, 2, 2, 2, 128], f8b)
            nc.sync.dma_start(out=wdrb_sb, in_=wdrb[:, :, :, :, :])
            vlo_sb = consts.tile([128, 1], f32r)
            nc.sync.dma_start(out=vlo_sb, in_=vlo[:, :])
            alph_sb = consts.tile([128, 1], f32)
            nc.sync.dma_start(out=alph_sb, in_=alph[:, :])
            w3t_sb = consts.tile([128, 2, 2, 128], bf16)
            nc.sync.dma_start(out=w3t_sb, in_=w3t[:, :, :, :])
            ht_sb = consts.tile([128, 2, BPC], bf16)
            nc.sync.dma_start(out=ht_sb, in_=ht[:, :, :])

            # Inline 0/1 masks for the softmax normalization matmuls:
            # bsum[b] = sum_n esums[8b+n]; brep[8b+n] = bsum[b].
            ma_np = np.zeros((64, BPC), np.float32)
            mb_np = np.zeros((BPC, 64), np.float32)
            for p in range(64):
                ma_np[p, p // NCH] = 1.0
                mb_np[p // NCH, p] = 1.0
            ma_dram = nc.inline_tensor(ma_np, name="ma")
            mb_dram = nc.inline_tensor(mb_np, name="mb")
            ma_sb = consts.tile([64, BPC], f32)
            nc.sync.dma_start(out=ma_sb, in_=ma_dram[:, :])
            mb_sb = consts.tile([BPC, 64], f32)
            nc.sync.dma_start(out=mb_sb, in_=mb_dram[:, :])

            # Per-batch bias: bias[m*128+c, b] = (W3 @ hidden[b])[m*128+c],
            # computed on device in bf16 (error ~1e-3 relative, negligible).
            bias_sb = consts.tile([128, 2, BPC], f32)
            for m in range(2):
                bias_ps = spsum.tile([128, BPC], f32, tag="small")
                for kk in range(2):
                    nc.tensor.matmul(
                        bias_ps,
                        lhsT=w3t_sb[:, kk, m, :],
                        rhs=ht_sb[:, kk, :],
                        start=(kk == 0),
                        stop=(kk == 1),
                    )
                nc.vector.tensor_copy(out=bias_sb[:, m, :], in_=bias_ps)

            loop_cm = (
                tc.For_i(0, loop_iters, 1) if loop_iters else contextlib.nullcontext()
            )
            with loop_cm:
              for _ in range(reps):
                # Scores live as [64, 512] with partition p = 8*b + n so the
                # epilogue runs on all 64 partitions at once.
                scores64 = spool.tile([64, 512], f32, tag="scores")
                pending = []

                def emit_vdot(pend):
                    row, vp, tc_ = pend
                    nc.tensor.matmul(
                        vp, lhsT=vlo_sb, rhs=tc_, start=True, stop=True
                    )
                    # Compute engines may only address partition bases
                    # that are multiples of 32, so the chunk is drained to
                    # partition 0 and a tiny SBUF->SBUF DMA places it at
                    # partition 8b+n of the scores tile.
                    stage = tpool.tile([1, 512], f32, tag="stage")
                    nc.vector.tensor_copy(out=stage, in_=vp)
                    nc.gpsimd.dma_start(
                        out=scores64[row : row + 1, :],
                        in_=stage,
                    )

                for b in range(BPC):
                    # Stream the two fp8 encoder tensors in 512 KiB halves;
                    # DRAM is already laid out in tile order so each DMA is
                    # a pure linear read.
                    xa, xb_ = {}, {}
                    for t in range(2):
                        xtile = xpool.tile([128, 2, SQ], f8a, tag=f"xa{t}")
                        nc.sync.dma_start(out=xtile, in_=xqa[b, t])
                        xa[t] = xtile
                        xtile = xpool.tile([128, 2, SQ], f8b, tag=f"xb{t}")
                        nc.sync.dma_start(out=xtile, in_=xqb[b, t])
                        xb_[t] = xtile

                    for n in range(NCH):
                        tt = tpool.tile([128, 2, 512], bf16, tag="tt")
                        for m in range(2):
                            ps = mpsum.tile([128, 512], f32, tag="ps")
                            if n < DRC:
                                r = n
                                for t in range(2):
                                    nc.tensor.matmul(
                                        ps,
                                        lhsT=wdra_sb[:, t, m],
                                        rhs=xa[t][:, :, r * 512 : (r + 1) * 512],
                                        start=(t == 0),
                                        stop=(t == 1),
                                        perf_mode=DR,
                                    )
                            else:
                                r = n - DRC
                                i = 0
                                for t in range(2):
                                    for kk in range(2):
                                        nc.tensor.matmul(
                                            ps,
                                            lhsT=wdrb_sb[:, t, m, kk, :],
                                            rhs=xb_[t][:, kk, r * 512 : (r + 1) * 512],
                                            start=(i == 0),
                                            stop=(i == 3),
                                        )
                                        i += 1
                            nc.scalar.activation(
                                out=tt[:, m, :],
                                in_=ps,
                                func=mybir.ActivationFunctionType.Tanh,
                                bias=bias_sb[:, m, b : b + 1],
                                scale=1.0 / SW,
                            )
                        # Fold the 256-partition v-reduction into 128 on the
                        # (idle) DVE: tc = t_lo + alpha * t_hi.
                        tc_ = tpool.tile([128, 512], f32r, tag="tc")
                        nc.vector.scalar_tensor_tensor(
                            out=tc_,
                            in0=tt[:, 1, :],
                            scalar=alph_sb[:, 0:1],
                            in1=tt[:, 0, :],
                            op0=mybir.AluOpType.mult,
                            op1=mybir.AluOpType.add,
                        )
                        vp = vpsum.tile([1, 512], f32, tag="vp")
                        pending.append((b * NCH + n, vp, tc_))
                        # v-dot runs two chunks late so the tanh+combine are
                        # ready and the PE never waits on ACT/DVE.
                        if len(pending) > 2:
                            emit_vdot(pending.pop(0))
                # flush the remaining v-dots
                for pend in pending:
                    emit_vdot(pend)
                pending = []

                # Softmax epilogue. Scores are small (|s| < ~6), so skip the
                # max subtraction: softmax = exp(s) / sum(exp(s)). The
                # per-batch sums are formed from the per-partition accum via
                # two tiny 0/1-mask matmuls (sum over n, then broadcast).
                esums = spool.tile([64, 1], f32, tag="esums")
                nc.scalar.activation(
                    out=scores64,
                    in_=scores64,
                    func=mybir.ActivationFunctionType.Exp,
                    accum_out=esums,
                )
                bsum_ps = spsum.tile([BPC, 1], f32, tag="small")
                nc.tensor.matmul(bsum_ps, lhsT=ma_sb, rhs=esums,
                                 start=True, stop=True)
                bsum_sb = spool.tile([BPC, 1], f32, tag="bsum")
                nc.vector.tensor_copy(out=bsum_sb, in_=bsum_ps)
                brep_ps = spsum.tile([64, 1], f32, tag="small")
                nc.tensor.matmul(brep_ps, lhsT=mb_sb, rhs=bsum_sb,
                                 start=True, stop=True)
                recip = spool.tile([64, 1], f32, tag="recip")
                nc.vector.reciprocal(out=recip, in_=brep_ps)
                nc.vector.tensor_scalar_mul(out=scores64, in0=scores64, scalar1=recip)
                nc.gpsimd.dma_start(
                    out=out[:, :].rearrange("b (n s) -> (b n) s", n=NCH),
                    in_=scores64,
                )

    nc.finalize()
    return nc


def prep_shared_inputs(W: np.ndarray, v: np.ndarray, decoder_hidden: np.ndarray):
    """Host-side layout marshaling of the small replicated parameters."""
    W = np.ascontiguousarray(W, dtype=np.float32)
    # wdr[p, t, m, i, c] = SW * W[m*128+c, t*H + i*128+p], quantized to fp8.
    wdra = np.empty((128, 2, 2, 2, 128), E4)
    wdrb = np.empty((128, 2, 2, 2, 128), E3)
    for t in range(2):
        Wt = W[:, t * H : (t + 1) * H]  # [h, k]
        for m in range(2):
            for i in range(2):
                blk = SW * Wt[m * 128 : (m + 1) * 128, i * 128 : (i + 1) * 128].T
                wdra[:, t, m, i, :] = blk.astype(E4)
                wdrb[:, t, m, i, :] = blk.astype(E3)
    vlo = np.ascontiguousarray(v[0][:128].reshape(128, 1), dtype=np.float32)
    alph = np.ascontiguousarray(
        (v[0][128:].astype(np.float64) / v[0][:128].astype(np.float64))
        .reshape(128, 1)
    ).astype(np.float32)
    # w3t[p, kk, m, c] = W3[m*128+c, kk*128+p] in bf16 (unscaled)
    W3 = W[:, 2 * H : 3 * H]  # [h, k]
    w3t = np.empty((128, 2, 2, 128), BF16)
    for kk in range(2):
        for m in range(2):
            w3t[:, kk, m, :] = (
                W3[m * 128 : (m + 1) * 128, kk * 128 : (kk + 1) * 128]
                .T.astype(BF16)
            )
    hT = decoder_hidden[0].T.astype(np.float32)  # [H, B]
    return wdra, wdrb, vlo, alph, w3t, hT


def _tileize(x: np.ndarray):
    """[B, H, S] fp32 -> ([B, 128, 2, SQ] e4m3 cols 0:SQ, [B, 128, 2, SQ] e3m4
    cols SQ:)."""
    xr = x.reshape(B, 2, 128, S)
    xa = np.ascontiguousarray(
        xr[:, :, :, :SQ].transpose(0, 2, 1, 3).astype(E4)
    )
    xb = np.ascontiguousarray(
        xr[:, :, :, SQ:].transpose(0, 2, 1, 3).astype(E3)
    )
    return xa, xb


_CACHED = {}


def _get_nc(reps: int = 1, loop_iters: int = 0):
    key = (reps, loop_iters)
    if key not in _CACHED:
        _CACHED[key] = build_bass(reps, loop_iters)
    return _CACHED[key]


def make_in_maps(static_enc, dynamic_enc, decoder_hidden, W, v):
    wdra, wdrb, vlo, alph, w3t, hT = prep_shared_inputs(W, v, decoder_hidden)
    xsa, xsb = _tileize(np.asarray(static_enc, dtype=np.float32))
    xda, xdb = _tileize(np.asarray(dynamic_enc, dtype=np.float32))
    # xq[b, t, p, i, s]
    xqa_all = np.ascontiguousarray(np.stack([xsa, xda], axis=1))
    xqb_all = np.ascontiguousarray(np.stack([xsb, xdb], axis=1))
    in_maps = []
    for c in range(N_CORES):
        b0 = c * BPC
        ht_c = np.ascontiguousarray(
            hT[:, b0 : b0 + BPC].reshape(2, 128, BPC).transpose(1, 0, 2)
        ).astype(BF16)  # [p, kk, b]
        in_maps.append(
            {
                "xqa": xqa_all[b0 : b0 + BPC],
                "xqb": xqb_all[b0 : b0 + BPC],
                "wdra": wdra,
                "wdrb": wdrb,
                "vlo": vlo,
                "alph": alph,
                "w3t": w3t,
                "ht": ht_c,
            }
        )
    return in_maps


def kernel(static_enc, dynamic_enc, decoder_hidden, W, v):
    from concourse.bass_utils import run_bass_kernel_spmd

    nc = _get_nc(reps=1)
    in_maps = make_in_maps(static_enc, dynamic_enc, decoder_hidden, W, v)
    res = run_bass_kernel_spmd(nc, in_maps, core_ids=list(range(N_CORES)))
    return np.concatenate([r["out"] for r in res.results], axis=0)


# revision 14
# speedup vs baseline: 1.7553x; 1.1389x over previous
"""Trainium2 Bass kernel for the sparse-attention scores module.

Computes, for each batch b:
    scores[b, :] = softmax_s( v . tanh(W1 @ static[b] + W2 @ dynamic[b] + W3 @ hidden[b]) )
with W = [W1 | W2 | W3] of shape [H, 3H], static/dynamic [B, H, S], hidden [B, H].

Sharding: data-parallel over B across 8 NeuronCores (8 batches per core).

v3: the two big encoder tensors are quantized to fp8 on the host and laid out
in DRAM in tile order (pure linear DMA, 16 MiB/core vs 64 for fp32). Output
columns are split by precision: the first DRC of 8 column chunks use fp8 e4m3
and run the [256,512] contraction in DoubleRow mode (2 fp8 weights per PE cell,
256-deep contraction per pass -> ~2x PE throughput); the rest use fp8 e3m4 (4
mantissa bits) at bf16 rate. The error contributions average across columns:
measured rel l2 vs the fp32 reference is ~1.6e-2 at DRC=4 (gate 2e-2).

The v-reduction uses the identity
    score[s] = sum_p v[p] * (t[p,s] + alpha[p] * t[128+p,s]),  alpha = v_hi/v_lo
so a single DVE multiply-add (on the otherwise idle Vector engine) folds the
256-partition reduction into 128, halving the v-dot matmul count on the PE.
W3 @ hidden is a tiny on-device bf16 matmul folded into the tanh via the ACT
per-partition bias; the fp8 weight scaling (x64) is undone by the ACT scale.
"""

import sys

sys.path.insert(0, "/opt/trn_rl_repo")

import numpy as np
import ml_dtypes

B, H, S = 64, 256, 4096
N_CORES = 8
BPC = B // N_CORES          # batches per core
NCH = S // 512              # 8 psum column chunks
DRC = 6                     # chunks (of 8) on the DoubleRow e4m3 path
SQA = DRC * 512             # e4m3 columns per batch
SQB = S - SQA               # e3m4 columns per batch
SW = 64.0                   # fp8 scale on W1/W2 (per-row refined by sig)
E4 = ml_dtypes.float8_e4m3  # TRN FP8_EXP4-compatible (max 240)
E3 = ml_dtypes.float8_e3m4
BF16 = ml_dtypes.bfloat16


def build_bass(reps: int = 1, loop_iters: int = 0):
    """Build the per-core Bass program. reps>1 unrolls the whole computation
    multiple times; loop_iters>0 additionally wraps the unrolled body in a
    hardware loop. Both are used only for timing by differencing."""
    import contextlib

    import concourse.bacc as bacc
    import concourse.tile as tile
    from concourse import mybir

    f32 = mybir.dt.float32
    f32r = mybir.dt.float32r
    bf16 = mybir.dt.bfloat16
    f8a = mybir.dt.float8e4
    f8b = mybir.dt.float8e3
    DR = mybir.MatmulPerfMode.DoubleRow

    nc = bacc.Bacc(None)

    xqa = nc.dram_tensor("xqa", [BPC, 2, 128, 2, SQA], f8a, kind="ExternalInput")
    xqb = nc.dram_tensor("xqb", [BPC, 2, 128, 2, SQB], f8b, kind="ExternalInput")
    wdra = nc.dram_tensor("wdra", [128, 2, 2, 2, 128], f8a, kind="ExternalInput")
    wdrb = nc.dram_tensor("wdrb", [128, 2, 2, 2, 128], f8b, kind="ExternalInput")
    scl = nc.dram_tensor("scl", [128, 2], f32, kind="ExternalInput")
    vlo = nc.dram_tensor("vlo", [128, 1], f32r, kind="ExternalInput")
    alph = nc.dram_tensor("alph", [128, 1], f32, kind="ExternalInput")
    w3t = nc.dram_tensor("w3t", [128, 2, 2, 128], bf16, kind="ExternalInput")
    ht = nc.dram_tensor("ht", [128, 2, BPC], bf16, kind="ExternalInput")
    out = nc.dram_tensor("out", [BPC, S], f32, kind="ExternalOutput")

    with tile.TileContext(nc) as tc:
        with (
            tc.tile_pool(name="consts", bufs=1) as consts,
            tc.tile_pool(name="xpool", bufs=2) as xpool,
            tc.tile_pool(name="tpool", bufs=6) as tpool,
            tc.tile_pool(name="spool", bufs=2) as spool,
            tc.tile_pool(name="mpsum", bufs=4, space="PSUM") as mpsum,
            tc.tile_pool(name="vpsum", bufs=2, space="PSUM") as vpsum,
            tc.tile_pool(name="spsum", bufs=2, space="PSUM") as spsum,
        ):
            wdra_sb = consts.tile([128, 2, 2, 2, 128], f8a)
            nc.sync.dma_start(out=wdra_sb, in_=wdra[:, :, :, :, :])
            wdrb_sb = consts.tile([128, 2, 2, 2, 128], f8b)
            nc.sync.dma_start(out=wdrb_sb, in_=wdrb[:, :, :, :, :])
            scl_sb = consts.tile([128, 2], f32)
            nc.sync.dma_start(out=scl_sb, in_=scl[:, :])
            vlo_sb = consts.tile([128, 1], f32r)
            nc.sync.dma_start(out=vlo_sb, in_=vlo[:, :])
            alph_sb = consts.tile([128, 1], f32)
            nc.sync.dma_start(out=alph_sb, in_=alph[:, :])
            w3t_sb = consts.tile([128, 2, 2, 128], bf16)
            nc.sync.dma_start(out=w3t_sb, in_=w3t[:, :, :, :])
            ht_sb = consts.tile([128, 2, BPC], bf16)
            nc.sync.dma_start(out=ht_sb, in_=ht[:, :, :])

            # Inline 0/1 masks for the softmax normalization matmuls:
            # bsum[b] = sum_n esums[8b+n]; brep[8b+n] = bsum[b].
            ma_np = np.zeros((64, BPC), np.float32)
            mb_np = np.zeros((BPC, 64), np.float32)
            for p in range(64):
                ma_np[p, p // NCH] = 1.0
                mb_np[p // NCH, p] = 1.0
            ma_dram = nc.inline_tensor(ma_np, name="ma")
            mb_dram = nc.inline_tensor(mb_np, name="mb")
            ma_sb = consts.tile([64, BPC], f32)
            nc.sync.dma_start(out=ma_sb, in_=ma_dram[:, :])
            mb_sb = consts.tile([BPC, 64], f32)
            nc.sync.dma_start(out=mb_sb, in_=mb_dram[:, :])

            # Per-batch bias: bias[m*128+c, b] = (W3 @ hidden[b])[m*128+c],
            # computed on device in bf16 (error ~1e-3 relative, negligible).
            bias_sb = consts.tile([128, 2, BPC], f32)
            for m in range(2):
                bias_ps = spsum.tile([128, BPC], f32, tag="small")
                for kk in range(2):
                    nc.tensor.matmul(
                        bias_ps,
                        lhsT=w3t_sb[:, kk, m, :],
                        rhs=ht_sb[:, kk, :],
                        start=(kk == 0),
                        stop=(kk == 1),
                    )
                nc.vector.tensor_copy(out=bias_sb[:, m, :], in_=bias_ps)

            loop_cm = (
                tc.For_i(0, loop_iters, 1) if loop_iters else contextlib.nullcontext()
            )
            with loop_cm:
              for _ in range(reps):
                # Scores live as [64, 512] with partition p = 8*b + n so the
                # epilogue runs on all 64 partitions at once.
                scores64 = spool.tile([64, 512], f32, tag="scores")
                pending = []

                def emit_vdot(pend):
                    row, vp, tc_ = pend
                    nc.tensor.matmul(
                        vp, lhsT=vlo_sb, rhs=tc_, start=True, stop=True
                    )
                    # Compute engines may only address partition bases
                    # that are multiples of 32, so the chunk is drained to
                    # partition 0 and a tiny SBUF->SBUF DMA places it at
                    # partition 8b+n of the scores tile.
                    stage = tpool.tile([1, 512], f32, tag="stage")
                    nc.vector.tensor_copy(out=stage, in_=vp)
                    nc.gpsimd.dma_start(
                        out=scores64[row : row + 1, :],
                        in_=stage,
                    )

                for b in range(BPC):
                    # Stream the two fp8 encoder tensors in 512 KiB halves;
                    # DRAM is already laid out in tile order so each DMA is
                    # a pure linear read.
                    xa, xb_ = {}, {}
                    for t in range(2):
                        xtile = xpool.tile([128, 2, SQA], f8a, tag=f"xa{t}")
                        nc.sync.dma_start(out=xtile, in_=xqa[b, t])
                        xa[t] = xtile
                        xtile = xpool.tile([128, 2, SQB], f8b, tag=f"xb{t}")
                        nc.sync.dma_start(out=xtile, in_=xqb[b, t])
                        xb_[t] = xtile

                    for n in range(NCH):
                        tt = tpool.tile([128, 2, 512], bf16, tag="tt")
                        for m in range(2):
                            ps = mpsum.tile([128, 512], f32, tag="ps")
                            if n < DRC:
                                r = n
                                for t in range(2):
                                    nc.tensor.matmul(
                                        ps,
                                        lhsT=wdra_sb[:, t, m],
                                        rhs=xa[t][:, :, r * 512 : (r + 1) * 512],
                                        start=(t == 0),
                                        stop=(t == 1),
                                        perf_mode=DR,
                                    )
                            else:
                                r = n - DRC
                                i = 0
                                for t in range(2):
                                    for kk in range(2):
                                        nc.tensor.matmul(
                                            ps,
                                            lhsT=wdrb_sb[:, t, m, kk, :],
                                            rhs=xb_[t][:, kk, r * 512 : (r + 1) * 512],
                                            start=(i == 0),
                                            stop=(i == 3),
                                        )
                                        i += 1
                            nc.scalar.activation(
                                out=tt[:, m, :],
                                in_=ps,
                                func=mybir.ActivationFunctionType.Tanh,
                                bias=bias_sb[:, m, b : b + 1],
                                scale=scl_sb[:, m : m + 1],
                            )
                        # Fold the 256-partition v-reduction into 128 on the
                        # (idle) DVE: tc = t_lo + alpha * t_hi.
                        tc_ = tpool.tile([128, 512], f32r, tag="tc")
                        nc.vector.scalar_tensor_tensor(
                            out=tc_,
                            in0=tt[:, 1, :],
                            scalar=alph_sb[:, 0:1],
                            in1=tt[:, 0, :],
                            op0=mybir.AluOpType.mult,
                            op1=mybir.AluOpType.add,
                        )
                        vp = vpsum.tile([1, 512], f32, tag="vp")
                        pending.append((b * NCH + n, vp, tc_))
                        # v-dot runs two chunks late so the tanh+combine are
                        # ready and the PE never waits on ACT/DVE.
                        if len(pending) > 2:
                            emit_vdot(pending.pop(0))
                # flush the remaining v-dots
                for pend in pending:
                    emit_vdot(pend)
                pending = []

                # Softmax epilogue. Scores are small (|s| < ~6), so skip the
                # max subtraction: softmax = exp(s) / sum(exp(s)). The
                # per-batch sums are formed from the per-partition accum via
                # two tiny 0/1-mask matmuls (sum over n, then broadcast).
                esums = spool.tile([64, 1], f32, tag="esums")
                nc.scalar.activation(
                    out=scores64,
                    in_=scores64,
                    func=mybir.ActivationFunctionType.Exp,
                    accum_out=esums,
                )
                bsum_ps = spsum.tile([BPC, 1], f32, tag="small")
                nc.tensor.matmul(bsum_ps, lhsT=ma_sb, rhs=esums,
                                 start=True, stop=True)
                bsum_sb = spool.tile([BPC, 1], f32, tag="bsum")
                nc.vector.tensor_copy(out=bsum_sb, in_=bsum_ps)
                brep_ps = spsum.tile([64, 1], f32, tag="small")
                nc.tensor.matmul(brep_ps, lhsT=mb_sb, rhs=bsum_sb,
                                 start=True, stop=True)
                recip = spool.tile([64, 1], f32, tag="recip")
                nc.vector.reciprocal(out=recip, in_=brep_ps)
                nc.vector.tensor_scalar_mul(out=scores64, in0=scores64, scalar1=recip)
                nc.gpsimd.dma_start(
                    out=out[:, :].rearrange("b (n s) -> (b n) s", n=NCH),
                    in_=scores64,
                )

    nc.finalize()
    return nc


def prep_shared_inputs(W: np.ndarray, v: np.ndarray, decoder_hidden: np.ndarray):
    """Host-side layout marshaling of the small replicated parameters."""
    W = np.ascontiguousarray(W, dtype=np.float32)
    # Per-row quantization scale search: for each output row h, pick the
    # scale in [1,2) that minimizes the e4m3 quantization error energy of
    # the actual [W1 | W2] row (the scale is undone by the tanh's ACT scale).
    W12 = np.concatenate([W[:, :H], W[:, H : 2 * H]], axis=1)  # [256, 512]
    best_sig = np.ones(H, np.float32)
    best_err = np.full(H, np.inf)
    for sg in np.exp2(np.linspace(0.0, 1.0, 33)[:-1]):
        Wq = (W12 * (SW * sg)).astype(E4).astype(np.float32) / (SW * sg)
        err = ((Wq - W12) ** 2).sum(axis=1)
        upd = err < best_err
        best_err[upd] = err[upd]
        best_sig[upd] = sg
    # wdr[p, t, m, i, c] = SW*sig[h] * W[h=m*128+c, t*H + i*128+p] in fp8.
    wdra = np.empty((128, 2, 2, 2, 128), E4)
    wdrb = np.empty((128, 2, 2, 2, 128), E3)
    for t in range(2):
        Wt = W[:, t * H : (t + 1) * H]  # [h, k]
        for m in range(2):
            sig_m = best_sig[m * 128 : (m + 1) * 128]  # [c]
            for i in range(2):
                blk = Wt[m * 128 : (m + 1) * 128, i * 128 : (i + 1) * 128].T
                blk = blk * (SW * sig_m[None, :])
                wdra[:, t, m, i, :] = blk.astype(E4)
                wdrb[:, t, m, i, :] = blk.astype(E3)
    # scl[p, m] = 1 / (SW * sig[m*128+p]) undoes the row scale pre-tanh.
    sclm = np.ascontiguousarray(
        (1.0 / (SW * best_sig.reshape(2, 128).T)), dtype=np.float32
    )
    vlo = np.ascontiguousarray(v[0][:128].reshape(128, 1), dtype=np.float32)
    alph = np.ascontiguousarray(
        (v[0][128:].astype(np.float64) / v[0][:128].astype(np.float64))
        .reshape(128, 1)
    ).astype(np.float32)
    # w3t[p, kk, m, c] = W3[m*128+c, kk*128+p] in bf16 (unscaled)
    W3 = W[:, 2 * H : 3 * H]  # [h, k]
    w3t = np.empty((128, 2, 2, 128), BF16)
    for kk in range(2):
        for m in range(2):
            w3t[:, kk, m, :] = (
                W3[m * 128 : (m + 1) * 128, kk * 128 : (kk + 1) * 128]
                .T.astype(BF16)
            )
    hT = decoder_hidden[0].T.astype(np.float32)  # [H, B]
    return wdra, wdrb, sclm, vlo, alph, w3t, hT


def _tileize(x: np.ndarray):
    """[B, H, S] fp32 -> ([B, 128, 2, SQA] e4m3 cols 0:SQA, [B, 128, 2, SQB]
    e3m4 cols SQA:)."""
    xr = x.reshape(B, 2, 128, S)
    xa = np.ascontiguousarray(
        xr[:, :, :, :SQA].transpose(0, 2, 1, 3).astype(E4)
    )
    xb = np.ascontiguousarray(
        xr[:, :, :, SQA:].transpose(0, 2, 1, 3).astype(E3)
    )
    return xa, xb


_CACHED = {}


def _get_nc(reps: int = 1, loop_iters: int = 0):
    key = (reps, loop_iters)
    if key not in _CACHED:
        _CACHED[key] = build_bass(reps, loop_iters)
    return _CACHED[key]


def make_in_maps(static_enc, dynamic_enc, decoder_hidden, W, v):
    wdra, wdrb, sclm, vlo, alph, w3t, hT = prep_shared_inputs(W, v, decoder_hidden)
    xsa, xsb = _tileize(np.asarray(static_enc, dtype=np.float32))
    xda, xdb = _tileize(np.asarray(dynamic_enc, dtype=np.float32))
    # xq[b, t, p, i, s]
    xqa_all = np.ascontiguousarray(np.stack([xsa, xda], axis=1))
    xqb_all = np.ascontiguousarray(np.stack([xsb, xdb], axis=1))
    in_maps = []
    for c in range(N_CORES):
        b0 = c * BPC
        ht_c = np.ascontiguousarray(
            hT[:, b0 : b0 + BPC].reshape(2, 128, BPC).transpose(1, 0, 2)
        ).astype(BF16)  # [p, kk, b]
        in_maps.append(
            {
                "xqa": xqa_all[b0 : b0 + BPC],
                "xqb": xqb_all[b0 : b0 + BPC],
                "wdra": wdra,
                "wdrb": wdrb,
                "scl": sclm,
                "vlo": vlo,
                "alph": alph,
                "w3t": w3t,
                "ht": ht_c,
            }
        )
    return in_maps


def kernel(static_enc, dynamic_enc, decoder_hidden, W, v):
    from concourse.bass_utils import run_bass_kernel_spmd

    nc = _get_nc(reps=1)
    in_maps = make_in_maps(static_enc, dynamic_enc, decoder_hidden, W, v)
    res = run_bass_kernel_spmd(nc, in_maps, core_ids=list(range(N_CORES)))
    return np.concatenate([r["out"] for r in res.results], axis=0)
